# revision 15
# baseline (speedup 1.0000x reference)
"""Bidirectional masked softmax geometric-mean kernel for Trainium2 (8 cores).

Problem: for each batch b (8 total):
  mask[i,j] = (i < L1_b) & (j < L2_b)
  logits    = where(mask, sim/TAU, -1e30)
  out       = where(mask, sqrt(EPS + softmax_row(logits) * softmax_col(logits)), 0)

Sharding: data-parallel over batch: core c handles slab c ([2048,2048] f32).

Math: with a fixed global stabilizer M (valid upper bound on logits),
  row_sm * col_sm = E^2 / (R_i * C_j),  E = exp(x/TAU - M),
  R_i = sum_j E (masked), C_j = sum_i E (masked)
so no per-row/col max pass is needed; exp underflow is benign because the
EPS floor dominates anything below 1e-8.

Device pipeline per 128-row tile (16 tiles):
  pass1: DVE add col-mask bias -> ACT exp (row bias; accum_out = row sums)
         -> PE per-128-col-block col-sum matmuls accumulated in PSUM [128,16]
  mid:   fixup+reciprocal of R and C in [128,16] layouts (128-lane),
         DRAM-bounce transpose of 1/C to a [1,2048] row, broadcast to [128,2048]
  pass2: ACT square -> DVE mul by 1/C -> ACT sqrt (scale=1/R_i, bias=EPS*rmask_i)
         -> GPSIMD mul by col mask -> DMA out
"""

import numpy as np
from contextlib import ExitStack

import concourse.bass as bass
import concourse.mybir as mybir
import concourse.tile as tile
from concourse.bass_utils import run_bass_kernel_spmd

B = 8
L = 2048
P = 128
NT = L // P  # 16 row tiles / col blocks
TAU = 0.5
EPS = 1e-8
MSTAB = 24.0  # global stabilizer in logit (x/TAU) units; logits are within ~±11
NEGB = 30000.0  # additive -inf substitute (exp underflows to exactly 0)
F32 = mybir.dt.float32

_CACHE = {}


HALF = 1024  # lengths are >= 1024, so columns [0, 1024) are always valid
CH = 512  # matmul free-dim chunk (PSUM bank limit)
NCH = L // CH  # 4 colsum accumulation chains


def _body(ctx, tc, x, cmask, auxT, crdram, cr2dram, y):
    nc = tc.nc
    Exp = mybir.ActivationFunctionType.Exp
    Sqrt = mybir.ActivationFunctionType.Sqrt
    mult = mybir.AluOpType.mult
    add = mybir.AluOpType.add

    singles = ctx.enter_context(tc.tile_pool(name="singles", bufs=1))
    xpool = ctx.enter_context(tc.tile_pool(name="xp", bufs=4))
    epool = ctx.enter_context(tc.tile_pool(name="ep", bufs=NT))
    pspool = ctx.enter_context(tc.tile_pool(name="ps", bufs=NCH, space="PSUM"))

    # --- constants / per-row vectors (right-half masks only; cols < 1024
    #     are always valid since L2 >= 1024) ---
    cmask_h = singles.tile([P, L - HALF], F32, tag="cmask_h")
    nc.sync.dma_start(out=cmask_h, in_=cmask[0:1, HALF:].to_broadcast([P, L - HALF]))
    ncmask_h = singles.tile([P, L - HALF], F32, tag="ncmask_h")
    nc.vector.tensor_scalar(ncmask_h, cmask_h, -1.0, 1.0, mult, add)  # 1 - cmask

    aux_sb = singles.tile([P, 4 * NT], F32, tag="aux")
    nc.sync.dma_start(out=aux_sb, in_=auxT[:, :])
    rbias_sb = aux_sb[:, 0:NT]
    sbias_sb = aux_sb[:, NT : 2 * NT]
    rfix_sb = aux_sb[:, 2 * NT : 3 * NT]
    cfix_sb = aux_sb[:, 3 * NT : 4 * NT]

    ones_sb = singles.tile([P, 1], F32, tag="ones")
    nc.vector.memset(ones_sb, 1.0)

    Rsum = singles.tile([P, NT], F32, tag="Rsum")
    Rbad = singles.tile([P, NT], F32, tag="Rbad")
    invR = singles.tile([P, NT], F32, tag="invR")
    Crow = singles.tile([1, L], F32, tag="Crow")
    CT = singles.tile([P, NT], F32, tag="CT")
    invCT = singles.tile([P, NT], F32, tag="invCT")
    invC_b = singles.tile([P, L], F32, tag="invC_b")

    E_tiles = [epool.tile([P, L], F32, tag="E", name=f"E{t}") for t in range(NT)]
    # 4 colsum accumulators [1, 512], one PSUM bank each; chain over t per chunk
    Cps = [pspool.tile([1, CH], F32, tag="Cps", name=f"Cps{c}") for c in range(NCH)]

    # --- pass 1: E = exp(2x + rbias) UNMASKED in columns (row masking via
    #     rbias). R = full rowsum (exp accum) minus the invalid-column tail
    #     (right-half STT accum). Colsums don't need column masking: invalid
    #     columns' C values are garbage but finite, and those outputs get
    #     zeroed by the final half-mask anyway. Then square E in place. ---
    for t in range(NT):
        xt = xpool.tile([P, L], F32, tag="xt")
        nc.sync.dma_start(out=xt, in_=x[t * P : (t + 1) * P, :])
        Et = E_tiles[t]
        nc.scalar.activation(
            Et,
            xt,
            Exp,
            bias=rbias_sb[:, t : t + 1],
            scale=2.0,
            accum_out=Rsum[:, t : t + 1],
        )
        # Rbad[:, t] = sum_j>=L2 E (output itself goes to dead xt space)
        nc.vector.scalar_tensor_tensor(
            xt[:, HALF:],
            Et[:, HALF:],
            1.0,
            ncmask_h,
            mult,
            mult,
            accum_out=Rbad[:, t : t + 1],
        )
        # colsum chains: Cps[c][0, :] += ones.T @ Et[:, chunk c]  (ones stationary)
        for c in range(NCH):
            nc.tensor.matmul(
                Cps[c][:, :],
                ones_sb,
                Et[:, c * CH : (c + 1) * CH],
                start=(t == 0),
                stop=(t == NT - 1),
            )
        # in-place square, alternating DVE/GPSIMD to balance engine load
        if t % 2 == 0:
            nc.vector.tensor_mul(Et, Et, Et)
        else:
            nc.gpsimd.tensor_mul(Et, Et, Et)

    # --- mid: reciprocals ---
    nc.vector.tensor_sub(Rsum, Rsum, Rbad)
    nc.vector.tensor_add(Rsum, Rsum, rfix_sb)
    nc.vector.reciprocal(invR, Rsum)

    # psum [1,512]x4 -> Crow [1,2048] -> dram -> CT [128,16] (16 parallel
    # column DMAs; CT[p, c] = C_j for j = c*128 + p) -> fixup -> reciprocal
    # -> dram (16 parallel DMAs) -> broadcast-read to invC_b [128, 2048].
    for c in range(NCH):
        nc.scalar.copy(Crow[0:1, c * CH : (c + 1) * CH], Cps[c][:, :])
    nc.sync.dma_start(out=crdram[0:1, :], in_=Crow)
    for k in range(NT):
        src = crdram[0:1, k * P : (k + 1) * P].rearrange("o p -> p o")
        nc.sync.dma_start(out=CT[:, k : k + 1], in_=src)
    nc.vector.tensor_add(CT, CT, cfix_sb)
    nc.vector.reciprocal(invCT, CT)
    for k in range(NT):
        dst = cr2dram[0:1, k * P : (k + 1) * P].rearrange("o p -> p o")
        nc.sync.dma_start(out=dst, in_=invCT[:, k : k + 1])
    nc.sync.dma_start(out=invC_b, in_=cr2dram[0:1, :].to_broadcast([P, L]))

    # --- pass 2: out = cmask * sqrt(E^2 * invC * invR + EPS*rmask) ---
    for t in range(NT):
        Et = E_tiles[t]  # holds E^2
        Pt = xpool.tile([P, L], F32, tag="xt")
        nc.vector.tensor_mul(Pt, Et, invC_b)
        nc.scalar.activation(
            Et, Pt, Sqrt, bias=sbias_sb[:, t : t + 1], scale=invR[:, t : t + 1]
        )
        nc.gpsimd.tensor_mul(Et[:, HALF:], Et[:, HALF:], cmask_h)
        nc.sync.dma_start(out=y[t * P : (t + 1) * P, :], in_=Et)


def _split_multi_waits(nc):
    """This walrus build's CoreV3 setupSyncWait rejects ANY instruction
    carrying more than one semaphore wait ("Too many sync wait commands");
    the ISA Events header has a single wait slot. Hoist extra waits onto
    preceding same-engine NoOps (sequential ge-waits on monotonic semaphores
    are equivalent to a combined wait). Apply only for the HW path — the
    synthetic NoOps lack the sim's sem bookkeeping and break CoreSim."""
    n = 0
    for fn in nc.m.functions:
        for bb in fn.blocks:
            out = []
            changed = False
            for inst in bb.instructions:
                si = inst.sync_info
                waits = list(si.on_wait) if (si and si.on_wait) else []
                if len(waits) > 1:
                    for w in waits[:-1]:
                        n += 1
                        out.append(
                            mybir.InstNoOp(
                                name=f"antsplitwait-{n}",
                                engine=inst.engine,
                                sync_info=mybir.SyncInfo(on_wait=[w], on_update=[]),
                            )
                        )
                    si.on_wait = waits[-1:]
                    changed = True
                out.append(inst)
            if changed:
                bb.instructions = out
    return nc


def build_nc(split_waits=True):
    nc = bass.Bass()
    x = nc.dram_tensor("x", [L, L], F32, kind="ExternalInput")
    cmask = nc.dram_tensor("cmask", [1, L], F32, kind="ExternalInput")
    auxT = nc.dram_tensor("auxT", [P, 4 * NT], F32, kind="ExternalInput")
    crdram = nc.dram_tensor("crscratch", [1, L], F32, kind="Internal")
    cr2dram = nc.dram_tensor("cr2scratch", [1, L], F32, kind="Internal")
    y = nc.dram_tensor("y", [L, L], F32, kind="ExternalOutput")

    with tile.TileContext(nc) as tc, ExitStack() as ctx:
        _body(ctx, tc, x, cmask, auxT, crdram, cr2dram, y)
    if split_waits:
        _split_multi_waits(nc)
    return nc


def get_nc():
    if "nc" not in _CACHE:
        _CACHE["nc"] = build_nc()
    return _CACHE["nc"]


def make_in_maps(sim_matrix, lengths):
    sim_matrix = np.ascontiguousarray(np.asarray(sim_matrix, dtype=np.float32))
    lengths = np.asarray(lengths, dtype=np.int32)
    idx = np.arange(L)
    in_maps = []
    for c in range(sim_matrix.shape[0]):
        l1, l2 = int(lengths[c, 0]), int(lengths[c, 1])
        rv = idx < l1  # row valid
        cv = idx < l2  # col valid

        def tcol(vals):  # [2048] -> [128, 16] with element i at [i%128, i//128]
            return np.ascontiguousarray(
                np.asarray(vals, dtype=np.float32).reshape(NT, P).T
            )

        auxT = np.concatenate(
            [
                tcol(np.where(rv, -MSTAB, -MSTAB - NEGB)),  # rbias
                tcol(np.where(rv, EPS, 0.0)),  # sbias
                tcol(np.where(rv, 0.0, 1.0)),  # rfix
                tcol(np.where(cv, 0.0, 1.0)),  # cfix
            ],
            axis=1,
        )
        in_maps.append(
            {
                "x": sim_matrix[c],
                "cmask": cv.astype(np.float32)[None, :],
                "auxT": np.ascontiguousarray(auxT),
            }
        )
    return in_maps


def run(sim_matrix, lengths, trace=False):
    nc = get_nc()
    in_maps = make_in_maps(sim_matrix, lengths)
    res = run_bass_kernel_spmd(nc, in_maps, list(range(B)), trace=trace)
    out = np.stack([res.results[c]["y"] for c in range(B)], axis=0)
    return out, res


def kernel(sim_matrix, lengths):
    out, _ = run(sim_matrix, lengths, trace=False)
    return out


# revision 18
# speedup vs baseline: 1.2788x; 1.2788x over previous
"""Bidirectional masked softmax geometric-mean kernel for Trainium2 (8 cores).

Problem: for each batch b (8 total):
  mask[i,j] = (i < L1_b) & (j < L2_b)
  logits    = where(mask, sim/TAU, -1e30)
  out       = where(mask, sqrt(EPS + softmax_row(logits) * softmax_col(logits)), 0)

Sharding: data-parallel over batch: core c handles slab c ([2048,2048] f32).

Math: with a fixed global stabilizer M (valid upper bound on logits),
  row_sm * col_sm = E^2 / (R_i * C_j),  E = exp(x/TAU - M),
  R_i = sum_j E (masked), C_j = sum_i E (masked)
so no per-row/col max pass is needed; exp underflow is benign because the
EPS floor dominates anything below 1e-8.

Device pipeline per 128-row tile (16 tiles):
  pass1: DVE add col-mask bias -> ACT exp (row bias; accum_out = row sums)
         -> PE per-128-col-block col-sum matmuls accumulated in PSUM [128,16]
  mid:   fixup+reciprocal of R and C in [128,16] layouts (128-lane),
         DRAM-bounce transpose of 1/C to a [1,2048] row, broadcast to [128,2048]
  pass2: ACT square -> DVE mul by 1/C -> ACT sqrt (scale=1/R_i, bias=EPS*rmask_i)
         -> GPSIMD mul by col mask -> DMA out
"""

import numpy as np
from contextlib import ExitStack

import concourse.bass as bass
import concourse.mybir as mybir
import concourse.tile as tile
from concourse.bass_utils import run_bass_kernel_spmd

B = 8
L = 2048
P = 128
NT = L // P  # 16 row tiles / col blocks
TAU = 0.5
EPS = 1e-8
MSTAB = 24.0  # global stabilizer in logit (x/TAU) units; logits are within ~±11
NEGB = 30000.0  # additive -inf substitute (exp underflows to exactly 0)
F32 = mybir.dt.float32

_CACHE = {}


HALF = 1024  # lengths are >= 1024, so columns [0, 1024) are always valid
CH = 512  # matmul free-dim chunk (PSUM bank limit)
NCH = L // CH  # 4 colsum accumulation chains


def _body(ctx, tc, x, cmask, auxT, crdram, y):
    nc = tc.nc
    Exp = mybir.ActivationFunctionType.Exp
    Sqrt = mybir.ActivationFunctionType.Sqrt
    mult = mybir.AluOpType.mult
    add = mybir.AluOpType.add

    singles = ctx.enter_context(tc.tile_pool(name="singles", bufs=1))
    xpool = ctx.enter_context(tc.tile_pool(name="xp", bufs=4))
    epool = ctx.enter_context(tc.tile_pool(name="ep", bufs=NT))
    pspool = ctx.enter_context(tc.tile_pool(name="ps", bufs=NCH, space="PSUM"))

    # --- constants / per-row vectors (right-half masks only; cols < 1024
    #     are always valid since L2 >= 1024) ---
    cmask_h = singles.tile([P, L - HALF], F32, tag="cmask_h")
    nc.sync.dma_start(out=cmask_h, in_=cmask[0:1, HALF:].to_broadcast([P, L - HALF]))
    ncmask_h = singles.tile([P, L - HALF], F32, tag="ncmask_h")
    nc.vector.tensor_scalar(ncmask_h, cmask_h, -1.0, 1.0, mult, add)  # 1 - cmask

    aux_sb = singles.tile([P, 4 * NT], F32, tag="aux")
    nc.sync.dma_start(out=aux_sb, in_=auxT[:, :])
    rbias_sb = aux_sb[:, 0:NT]
    sbias_sb = aux_sb[:, NT : 2 * NT]
    rfix_sb = aux_sb[:, 2 * NT : 3 * NT]
    cfix_sb = aux_sb[:, 3 * NT : 4 * NT]

    ones_sb = singles.tile([P, 1], F32, tag="ones")
    nc.vector.memset(ones_sb, 1.0)

    Rsum = singles.tile([P, NT], F32, tag="Rsum")
    Rbad = singles.tile([P, NT], F32, tag="Rbad")
    invR = singles.tile([P, NT], F32, tag="invR")
    Crow = singles.tile([1, L], F32, tag="Crow")
    invC_b = singles.tile([P, L], F32, tag="invC_b")

    E_tiles = [epool.tile([P, L], F32, tag="E", name=f"E{t}") for t in range(NT)]
    # 4 colsum accumulators [1, 512], one PSUM bank each; chain over t per chunk
    Cps = [pspool.tile([1, CH], F32, tag="Cps", name=f"Cps{c}") for c in range(NCH)]

    # --- pass 1: E = exp(2x + rbias) UNMASKED in columns (row masking via
    #     rbias). R = full rowsum (exp accum) minus the invalid-column tail
    #     (right-half STT accum). Colsums don't need column masking: invalid
    #     columns' C values are garbage but finite, and those outputs get
    #     zeroed by the final half-mask anyway. Then square E in place. ---
    for t in range(NT):
        xt = xpool.tile([P, L], F32, tag="xt")
        nc.sync.dma_start(out=xt, in_=x[t * P : (t + 1) * P, :])
        Et = E_tiles[t]
        nc.scalar.activation(
            Et,
            xt,
            Exp,
            bias=rbias_sb[:, t : t + 1],
            scale=2.0,
            accum_out=Rsum[:, t : t + 1],
        )
        # Rbad[:, t] = sum_j>=L2 E (output itself goes to dead xt space)
        nc.vector.scalar_tensor_tensor(
            xt[:, HALF:],
            Et[:, HALF:],
            1.0,
            ncmask_h,
            mult,
            mult,
            accum_out=Rbad[:, t : t + 1],
        )
        # colsum chains: Cps[c][0, :] += ones.T @ Et[:, chunk c]  (ones stationary)
        for c in range(NCH):
            nc.tensor.matmul(
                Cps[c][:, :],
                ones_sb,
                Et[:, c * CH : (c + 1) * CH],
                start=(t == 0),
                stop=(t == NT - 1),
            )
        # in-place square, spread over ACT/DVE/GPSIMD to balance engine load
        if t % 3 == 0:
            nc.scalar.activation(Et, Et, mybir.ActivationFunctionType.Square)
        elif t % 3 == 1:
            nc.vector.tensor_mul(Et, Et, Et)
        else:
            nc.gpsimd.tensor_mul(Et, Et, Et)

    # --- mid ---
    nc.vector.tensor_sub(Rsum, Rsum, Rbad)
    nc.vector.tensor_add(Rsum, Rsum, rfix_sb)
    nc.vector.reciprocal(invR, Rsum)

    # invC without any transposes: psum [1,512]x4 -> Crow [1,2048] -> dram ->
    # broadcast-read to [128,2048], then invC = exp(-ln(C)) on ACT (ln and
    # exp share the natural_log_exp table set). C > 0 always (unmasked
    # column sums of exps), so ln is safe; invalid columns give garbage but
    # finite invC, and the final half-mask zeroes those outputs anyway.
    for c in range(NCH):
        nc.scalar.copy(Crow[0:1, c * CH : (c + 1) * CH], Cps[c][:, :])
    nc.sync.dma_start(out=crdram[0:1, :], in_=Crow)
    nc.sync.dma_start(out=invC_b, in_=crdram[0:1, :].to_broadcast([P, L]))
    nc.scalar.activation(invC_b, invC_b, mybir.ActivationFunctionType.Ln)
    nc.scalar.activation(invC_b, invC_b, Exp, scale=-1.0)

    # --- pass 2: out = cmask * sqrt(E^2 * invC * invR + EPS*rmask) ---
    for t in range(NT):
        Et = E_tiles[t]  # holds E^2
        Pt = xpool.tile([P, L], F32, tag="xt")
        nc.vector.tensor_mul(Pt, Et, invC_b)
        nc.scalar.activation(
            Et, Pt, Sqrt, bias=sbias_sb[:, t : t + 1], scale=invR[:, t : t + 1]
        )
        if t % 2 == 0:
            nc.gpsimd.tensor_mul(Et[:, HALF:], Et[:, HALF:], cmask_h)
        else:
            nc.vector.tensor_mul(Et[:, HALF:], Et[:, HALF:], cmask_h)
        nc.sync.dma_start(out=y[t * P : (t + 1) * P, :], in_=Et)


def _split_multi_waits(nc):
    """This walrus build's CoreV3 setupSyncWait rejects ANY instruction
    carrying more than one semaphore wait ("Too many sync wait commands");
    the ISA Events header has a single wait slot. Hoist extra waits onto
    preceding same-engine NoOps (sequential ge-waits on monotonic semaphores
    are equivalent to a combined wait). Apply only for the HW path — the
    synthetic NoOps lack the sim's sem bookkeeping and break CoreSim."""
    n = 0
    for fn in nc.m.functions:
        for bb in fn.blocks:
            out = []
            changed = False
            for inst in bb.instructions:
                si = inst.sync_info
                waits = list(si.on_wait) if (si and si.on_wait) else []
                if len(waits) > 1:
                    for w in waits[:-1]:
                        n += 1
                        out.append(
                            mybir.InstNoOp(
                                name=f"antsplitwait-{n}",
                                engine=inst.engine,
                                sync_info=mybir.SyncInfo(on_wait=[w], on_update=[]),
                            )
                        )
                    si.on_wait = waits[-1:]
                    changed = True
                out.append(inst)
            if changed:
                bb.instructions = out
    return nc


def build_nc(split_waits=True):
    nc = bass.Bass()
    x = nc.dram_tensor("x", [L, L], F32, kind="ExternalInput")
    cmask = nc.dram_tensor("cmask", [1, L], F32, kind="ExternalInput")
    auxT = nc.dram_tensor("auxT", [P, 4 * NT], F32, kind="ExternalInput")
    crdram = nc.dram_tensor("crscratch", [1, L], F32, kind="Internal")
    y = nc.dram_tensor("y", [L, L], F32, kind="ExternalOutput")

    with tile.TileContext(nc) as tc, ExitStack() as ctx:
        _body(ctx, tc, x, cmask, auxT, crdram, y)
    if split_waits:
        _split_multi_waits(nc)
    return nc


def get_nc():
    if "nc" not in _CACHE:
        _CACHE["nc"] = build_nc()
    return _CACHE["nc"]


def make_in_maps(sim_matrix, lengths):
    sim_matrix = np.ascontiguousarray(np.asarray(sim_matrix, dtype=np.float32))
    lengths = np.asarray(lengths, dtype=np.int32)
    idx = np.arange(L)
    in_maps = []
    for c in range(sim_matrix.shape[0]):
        l1, l2 = int(lengths[c, 0]), int(lengths[c, 1])
        rv = idx < l1  # row valid
        cv = idx < l2  # col valid

        def tcol(vals):  # [2048] -> [128, 16] with element i at [i%128, i//128]
            return np.ascontiguousarray(
                np.asarray(vals, dtype=np.float32).reshape(NT, P).T
            )

        auxT = np.concatenate(
            [
                tcol(np.where(rv, -MSTAB, -MSTAB - NEGB)),  # rbias
                tcol(np.where(rv, EPS, 0.0)),  # sbias
                tcol(np.where(rv, 0.0, 1.0)),  # rfix
                tcol(np.where(cv, 0.0, 1.0)),  # cfix
            ],
            axis=1,
        )
        in_maps.append(
            {
                "x": sim_matrix[c],
                "cmask": cv.astype(np.float32)[None, :],
                "auxT": np.ascontiguousarray(auxT),
            }
        )
    return in_maps


def run(sim_matrix, lengths, trace=False):
    nc = get_nc()
    in_maps = make_in_maps(sim_matrix, lengths)
    res = run_bass_kernel_spmd(nc, in_maps, list(range(B)), trace=trace)
    out = np.stack([res.results[c]["y"] for c in range(B)], axis=0)
    return out, res


def kernel(sim_matrix, lengths):
    out, _ = run(sim_matrix, lengths, trace=False)
    return out
